# revision 4
# baseline (speedup 1.0000x reference)
"""Trainium2 Bass kernel for a 2-layer GraphConv GNN + mean-pool + linear.

Reference computation (all fp32):
    h1 = leaky_relu(segsum(w*x[src] -> dst) @ W1_rel + x @ W1_root + b1)
    h2 = leaky_relu(segsum(w*h1[src] -> dst) @ W2_rel + h1 @ W2_root + b2)
    pooled = segment_mean(h2, batch, 512)
    out = pooled @ Wl_root + bl            # [512, 8]

Distribution (8 NeuronCores): nodes sharded in contiguous ranges of 12500
per core; edges live on the core owning their dst node.

Implementation notes:
  - Scatter-add over edges is a one-hot matmul per 128-edge chunk: DVE
    builds onehot[e, d] = (d == dib_e) * w_e in bf16, TensorE contracts
    gathered-src-rows x onehot into a feature-major PSUM tile.
  - Layer-1 edge "gathers" are done on the HOST: src indices are static,
    so the per-edge x rows are pre-laid-out as a contiguous bf16 stream
    and DMA-loaded at full bandwidth (no per-row descriptors).
  - Layer-2 rows come from the device-computed h1, fetched with batched
    SWDGE dma_gather spread over all 4 GPSIMD queue pairs (desc-gen
    throughput is the limit, ~3ns/row).  dma_gather indices are int16, so
    the h1 table is split into 4 banks of <32768 rows; each core's edge
    stream is sorted (group, bank, dst-block) and padded per (block,
    bank) to 128-edge chunks.  The table stores each row twice
    ([h, h] bf16 = 256B) to satisfy dma_gather's 256B elem/stride
    granularity.
  - All matmuls run in bf16 (4x faster than f32 on PE); PSUM accumulates
    fp32.  leaky_relu = relu(z+b) - relu(-s*(z+b)): both relu terms on
    the Scalar engine, DVE only does a tensor_tensor subtract.
  - h1 shards are exchanged with an AllGather; per-graph pooling is a
    one-hot matmul accumulated in PSUM and finished (overlap-add + mean
    + final 64x8 linear) on host.
"""

import math

import ml_dtypes
import numpy as np

import concourse.bacc as bacc
import concourse.bass as bass
import concourse.mybir as mybir
import concourse.tile as tile
from concourse.bass_utils import run_bass_kernel_spmd

F32 = mybir.dt.float32
BF16 = mybir.dt.bfloat16
I16 = mybir.dt.int16
ALU = mybir.AluOpType
ACTF = mybir.ActivationFunctionType
BF = ml_dtypes.bfloat16

N, E, D, NGRAPH, CORES = 100000, 1250000, 64, 512, 8
NPC = N // CORES                    # 12500 nodes per core
BLOCK = 128
NB = math.ceil(NPC / BLOCK)         # 98 dst blocks per core
NPAD = NB * BLOCK                   # 12544
GROUP = 7                           # dst blocks per gather group
NG = NB // GROUP                    # 14 groups
NBANK = 4
HALF = NPAD // 2                    # 6272: blocks 0-48 -> A, 49-97 -> B
TBL = HALF * CORES                  # 50176 rows per half-table
BK2 = TBL // 2                      # 25088 rows per h1-table bank
NQ = 4                              # SWDGE queues for dma_gather
LEAKY = 0.01


# ---------------------------------------------------------------------------
# Host-side preprocessing
# ---------------------------------------------------------------------------

def _make_stream(core_of, blk, dib, w, ids, BK, nbank):
    """Chunked edge-stream layout for one layer.

    Edges (already dst-sorted) are ordered (core, group, bank, block) and
    padded per (block, bank) to 128-edge chunks with a shared-across-cores
    chunk count.  Returns the static layout (K, calls, col_of, C), the
    per-core dib/wgt arrays, and the per-edge (core, position) assignment
    so the caller can lay out idx or data streams.
    """
    bank = ids // BK if nbank > 1 else np.zeros_like(ids)
    loc = ids % BK if nbank > 1 else ids

    key = (core_of * NB + blk) * nbank + bank
    cnt = np.bincount(key, minlength=CORES * NB * nbank)
    cnt = cnt.reshape(CORES, NB, nbank)
    K = -(-cnt.max(axis=0) // BLOCK)             # [NB, nbank]
    empty = K.sum(axis=1) == 0
    K[empty, 0] = 1

    col_of = np.zeros((NB, nbank), np.int64)
    calls = []                                   # (g, k, col0, Kgk)
    c = 0
    for g in range(NG):
        bs = range(g * GROUP, (g + 1) * GROUP)
        for k in range(nbank):
            c0 = c
            for b in bs:
                col_of[b, k] = c
                c += K[b, k]
            calls.append((g, k, c0, c - c0))
    C = c

    g_of = blk // GROUP
    order = np.lexsort((blk, bank, g_of, core_of))
    ck = key[order]
    first = np.concatenate(([True], ck[1:] != ck[:-1]))
    run_start_idx = np.flatnonzero(first)
    run_id = np.cumsum(first) - 1
    rank = np.arange(len(ck)) - run_start_idx[run_id]
    pos = (col_of[blk[order], bank[order]] + rank // BLOCK) * BLOCK \
        + rank % BLOCK

    dib_arr = np.full((CORES, BLOCK, C), -1.0, np.float32)
    wgt_arr = np.zeros((CORES, BLOCK, C), np.float32)
    cs = core_of[order]
    dib_arr[cs, pos % BLOCK, pos // BLOCK] = dib[order]
    wgt_arr[cs, pos % BLOCK, pos // BLOCK] = w[order]

    return dict(K=K, calls=calls, col_of=col_of, C=C,
                dib=np.ascontiguousarray(dib_arr),
                wgt=np.ascontiguousarray(wgt_arr),
                order=order, pos=pos, cs=cs, loc=loc)


def preprocess(x, edge_index, weights, batch):
    src = np.asarray(edge_index[0], dtype=np.int64)
    dst = np.asarray(edge_index[1], dtype=np.int64)
    w = np.asarray(weights, dtype=np.float32)
    batch = np.asarray(batch, dtype=np.int64)

    order = np.argsort(dst, kind="stable")
    src, dst, w = src[order], dst[order], w[order]
    core_of = dst // NPC
    ld = dst - core_of * NPC
    blk = ld // BLOCK
    dib = (ld % BLOCK).astype(np.float32)

    xbf = x.astype(BF)

    # Layer 1: host-side pre-gather into a contiguous edge-major stream.
    st1 = _make_stream(core_of, blk, dib, w, src, N, 1)
    C1 = st1["C"]
    xg = np.zeros((CORES, BLOCK, C1, D), BF)
    pos1, cs1 = st1["pos"], st1["cs"]
    xg[cs1, pos1 % BLOCK, pos1 // BLOCK] = xbf[src[st1["order"]]]
    xg = np.ascontiguousarray(xg.reshape(CORES, BLOCK, C1 * D))

    # Layer 2: device gathers from the two half h1 tables (A: local node
    # slot < HALF i.e. dst blocks 0-48 of the owner; B: the rest).
    owner = src // NPC
    local = src % NPC
    id2 = np.where(local < HALF,
                   owner * HALF + local,
                   TBL + owner * HALF + (local - HALF))
    st2 = _make_stream(core_of, blk, dib, w, id2, BK2, NBANK)
    C2 = st2["C"]
    idx2 = np.zeros((CORES, C2 * BLOCK), np.int16)
    idx2[st2["cs"], st2["pos"]] = st2["loc"][st2["order"]].astype(np.int16)
    idx2_w = np.stack([
        np.ascontiguousarray(
            np.tile(idx2[cc].reshape(C2 * 8, 16).T, (8, 1)))
        for cc in range(CORES)])                 # [CORES, 128, C2*8]

    g_base = batch[np.arange(CORES) * NPC]
    in_maps = []
    for c in range(CORES):
        xT = np.zeros((D, NPAD), BF)
        xT[:, :NPC] = xbf[c * NPC:(c + 1) * NPC].T
        gs = np.full(NPAD, -1.0, np.float32)
        gs[:NPC] = (batch[c * NPC:(c + 1) * NPC] - g_base[c]).astype(
            np.float32)
        assert gs.max() < 128.0, "graph span per core exceeds 128"
        in_maps.append({
            "xg": xg[c],
            "xT": np.ascontiguousarray(xT),
            "dib1": st1["dib"][c], "wgt1": st1["wgt"][c],
            "idx2": idx2_w[c], "dib2": st2["dib"][c],
            "wgt2": st2["wgt"][c],
            "gslot": np.ascontiguousarray(gs.reshape(NB, BLOCK).T),
        })
    meta1 = {k: st1[k] for k in ("K", "calls", "col_of", "C")}
    meta2 = {k: st2[k] for k in ("K", "calls", "col_of", "C")}
    return in_maps, meta1, meta2, g_base


def _common_inputs(W1_root, W1_rel, b1, W2_root, W2_rel, b2):
    return {
        "w1r": np.asarray(W1_rel, np.float32).astype(BF),
        "w1o": np.asarray(W1_root, np.float32).astype(BF),
        "w2r": np.asarray(W2_rel, np.float32).astype(BF),
        "w2o": np.asarray(W2_root, np.float32).astype(BF),
        "b1": np.ascontiguousarray(
            np.asarray(b1, np.float32).reshape(D, 1)),
        "nb1": np.ascontiguousarray(
            (-LEAKY * np.asarray(b1, np.float32)).reshape(D, 1)),
        "b2": np.ascontiguousarray(
            np.asarray(b2, np.float32).reshape(D, 1)),
        "nb2": np.ascontiguousarray(
            (-LEAKY * np.asarray(b2, np.float32)).reshape(D, 1)),
        "iota": np.broadcast_to(np.arange(BLOCK, dtype=np.float32),
                                (BLOCK, BLOCK)).astype(BF).copy(),
        "idm": np.eye(D, dtype=np.float32).astype(BF),
    }


# ---------------------------------------------------------------------------
# Bass program
# ---------------------------------------------------------------------------

def build_nc(meta1, meta2, repeat=1, debug_h1=False, parts="full"):
    C1, C2 = meta1["C"], meta2["C"]
    KG1 = max(sum(Kgk for (gg, k, c0, Kgk) in meta1["calls"] if gg == g)
              for g in range(NG))
    KG2A = max(sum(Kgk for (gg, k, c0, Kgk) in meta2["calls"]
                   if gg == g and k < 2) for g in range(NG))
    KG2B = max(sum(Kgk for (gg, k, c0, Kgk) in meta2["calls"]
                   if gg == g and k >= 2) for g in range(NG))

    nc = bacc.Bacc("TRN2", target_bir_lowering=False, debug=False,
                   num_devices=CORES, num_swdge_queues=NQ)

    xg_d = nc.dram_tensor("xg", [128, C1 * D], BF16, kind="ExternalInput")
    xT_d = nc.dram_tensor("xT", [D, NPAD], BF16, kind="ExternalInput")
    dib1_d = nc.dram_tensor("dib1", [128, C1], F32, kind="ExternalInput")
    wgt1_d = nc.dram_tensor("wgt1", [128, C1], F32, kind="ExternalInput")
    idx2_d = nc.dram_tensor("idx2", [128, C2 * 8], I16, kind="ExternalInput")
    dib2_d = nc.dram_tensor("dib2", [128, C2], F32, kind="ExternalInput")
    wgt2_d = nc.dram_tensor("wgt2", [128, C2], F32, kind="ExternalInput")
    gslot_d = nc.dram_tensor("gslot", [128, NB], F32, kind="ExternalInput")
    w1r_d = nc.dram_tensor("w1r", [D, D], BF16, kind="ExternalInput")
    w1o_d = nc.dram_tensor("w1o", [D, D], BF16, kind="ExternalInput")
    w2r_d = nc.dram_tensor("w2r", [D, D], BF16, kind="ExternalInput")
    w2o_d = nc.dram_tensor("w2o", [D, D], BF16, kind="ExternalInput")
    b1_d = nc.dram_tensor("b1", [D, 1], F32, kind="ExternalInput")
    nb1_d = nc.dram_tensor("nb1", [D, 1], F32, kind="ExternalInput")
    b2_d = nc.dram_tensor("b2", [D, 1], F32, kind="ExternalInput")
    nb2_d = nc.dram_tensor("nb2", [D, 1], F32, kind="ExternalInput")
    iota_d = nc.dram_tensor("iota", [128, 128], BF16, kind="ExternalInput")
    idm_d = nc.dram_tensor("idm", [D, D], BF16, kind="ExternalInput")

    pool_d = nc.dram_tensor("pool", [128, D], F32, kind="ExternalOutput")
    if debug_h1:
        h1dbg_d = nc.dram_tensor("h1dbg", [NPAD, 2 * D], BF16,
                                 kind="ExternalOutput")

    h1dlA = nc.dram_tensor("h1dlA", [HALF, 2 * D], BF16)
    h1dlB = nc.dram_tensor("h1dlB", [HALF, 2 * D], BF16)
    h1dfA = nc.dram_tensor("h1dfA", [TBL, 2 * D], BF16, addr_space="Shared")
    h1dfB = nc.dram_tensor("h1dfB", [TBL, 2 * D], BF16, addr_space="Shared")

    with tile.TileContext(nc) as tc:
        with (
            tc.tile_pool(name="persist", bufs=1) as pp,
            tc.tile_pool(name="gat", bufs=2) as gp,
            tc.tile_pool(name="gatB", bufs=2) as gpb,
            tc.tile_pool(name="gatA", bufs=4) as gpa,
            tc.tile_pool(name="oh", bufs=6) as ohp,
            tc.tile_pool(name="work", bufs=4) as wp,
            tc.tile_pool(name="aggps", bufs=2, space="PSUM") as aggp,
            tc.tile_pool(name="zps", bufs=2, space="PSUM") as zp,
            tc.tile_pool(name="tps", bufs=2, space="PSUM") as tpp,
            tc.tile_pool(name="plps", bufs=1, space="PSUM") as plp,
        ):
            h1T_s = pp.tile([D, NPAD], BF16, tag="h1T")
            dib1_s = pp.tile([128, C1], F32, tag="dib1")
            wgt1_s = pp.tile([128, C1], F32, tag="wgt1")
            idx2_s = pp.tile([128, C2 * 8], I16, tag="idx2")
            dib2_s = pp.tile([128, C2], F32, tag="dib2")
            wgt2_s = pp.tile([128, C2], F32, tag="wgt2")
            gslot_s = pp.tile([128, NB], F32, tag="gslot")
            w1r_s = pp.tile([D, D], BF16, tag="w1r")
            w1o_s = pp.tile([D, D], BF16, tag="w1o")
            w2r_s = pp.tile([D, D], BF16, tag="w2r")
            w2o_s = pp.tile([D, D], BF16, tag="w2o")
            b1_s = pp.tile([D, 1], F32, tag="b1")
            nb1_s = pp.tile([D, 1], F32, tag="nb1")
            b2_s = pp.tile([D, 1], F32, tag="b2")
            nb2_s = pp.tile([D, 1], F32, tag="nb2")
            iota_s = pp.tile([128, 128], BF16, tag="iota")
            idm_s = pp.tile([D, D], BF16, tag="idm")

            for t, d in [(dib1_s, dib1_d), (wgt1_s, wgt1_d),
                         (idx2_s, idx2_d), (dib2_s, dib2_d),
                         (wgt2_s, wgt2_d), (gslot_s, gslot_d),
                         (w1r_s, w1r_d), (w1o_s, w1o_d),
                         (w2r_s, w2r_d), (w2o_s, w2o_d), (b1_s, b1_d),
                         (nb1_s, nb1_d), (b2_s, b2_d), (nb2_s, nb2_d),
                         (iota_s, iota_d), (idm_s, idm_d)]:
                nc.sync.dma_start(out=t[:], in_=d[:, :])

            pool_ps = plp.tile([128, D], F32, tag="pool")

            def compute_group(meta, g, dib_s, wgt_s, xfm_of, wr_s, wo_s,
                              tail, chunk_lhsT):
                """chunk_lhsT(bank, col) -> lhsT AP for that chunk, where
                col is the bank-relative chunk column within the group."""
                K = meta["K"]
                nbank = K.shape[1]
                col_of = meta["col_of"]
                bs = list(range(g * GROUP, (g + 1) * GROUP))
                for b in bs:
                    total = int(K[b].sum())
                    agg = aggp.tile([D, 128], F32, tag="agg")
                    done = 0
                    for k in range(nbank):
                        colk = int(sum(K[b2][k] for b2 in bs if b2 < b))
                        for j in range(int(K[b][k])):
                            cg = int(col_of[b][k]) + j
                            oht = ohp.tile([128, 128], BF16, tag="oh")
                            nc.vector.tensor_scalar(
                                out=oht[:], in0=iota_s[:],
                                scalar1=dib_s[:, cg:cg + 1],
                                scalar2=wgt_s[:, cg:cg + 1],
                                op0=ALU.is_equal, op1=ALU.mult)
                            nc.tensor.matmul(
                                out=agg[:],
                                lhsT=chunk_lhsT(k, colk + j),
                                rhs=oht[:],
                                start=(done == 0),
                                stop=(done == total - 1))
                            done += 1
                    aggs = wp.tile([D, 128], BF16, tag="aggs")
                    nc.scalar.activation(out=aggs[:], in_=agg[:],
                                         func=ACTF.Copy)
                    z = zp.tile([D, 128], F32, tag="z")
                    nc.tensor.matmul(out=z[:], lhsT=wr_s[:], rhs=aggs[:],
                                     start=True, stop=False)
                    nc.tensor.matmul(
                        out=z[:], lhsT=wo_s[:],
                        rhs=xfm_of(b),
                        start=False, stop=True)
                    tail(b, z)

            # ---- layer-1 group fetch: contiguous stream load ----
            call1 = {(g, k): (c0, Kgk)
                     for (g, k, c0, Kgk) in meta1["calls"]}
            call2 = {(g, k): (c0, Kgk)
                     for (g, k, c0, Kgk) in meta2["calls"]}
            gq = [0]

            def fetch1(g):
                c0, Kg = call1[(g, 0)]
                t = gp.tile([128, KG1 * D], BF16, tag="xgt")
                nc.sync.dma_start(
                    out=t[:, 0:Kg * D],
                    in_=xg_d[:, c0 * D:(c0 + Kg) * D])
                xt = gp.tile([D, GROUP * 128], BF16, tag="xtg")
                nc.sync.dma_start(
                    out=xt[:],
                    in_=xT_d[:, g * GROUP * 128:(g + 1) * GROUP * 128])
                return t, xt

            def lhsT1(handle, col):
                return handle[:, col * D:(col + 1) * D]

            def lhsT2(handle, col):
                return handle[:, col, 0:D]

            def gather_half(g, half):
                """Emit the dma_gather calls for group g, banks of one half
                (half 0 -> banks 0,1 from h1dfA; half 1 -> banks 2,3 from
                h1dfB).  Returns (tile, {bank: col offset})."""
                tab = h1dfA if half == 0 else h1dfB
                kg = KG2A if half == 0 else KG2B
                t = (gpa if half == 0 else gpb).tile(
                    [128, kg, 128], BF16, tag=f"gt{half}")
                off = {}
                o = 0
                for k in (0, 1):
                    kk = half * 2 + k
                    c0, Kgk = call2[(g, kk)]
                    off[kk] = o
                    if Kgk == 0:
                        continue
                    nc.gpsimd.dma_gather(
                        t[:, o:o + Kgk, :],
                        tab[k * BK2:(k + 1) * BK2, :],
                        idx2_s[:, c0 * 8:(c0 + Kgk) * 8],
                        Kgk * 128, Kgk * 128, 128, elem_step=128,
                        single_packet=False, queue_num=gq[0] % NQ)
                    gq[0] += 1
                    o += Kgk
                return t, off

            def l1_tail(b, z):
                a1 = wp.tile([D, 128], BF16, tag="a1")
                nc.scalar.activation(out=a1[:], in_=z[:], func=ACTF.Relu,
                                     bias=b1_s[:, 0:1], scale=1.0)
                a2 = wp.tile([D, 128], BF16, tag="a2")
                nc.scalar.activation(out=a2[:], in_=z[:], func=ACTF.Relu,
                                     bias=nb1_s[:, 0:1], scale=-LEAKY)
                hsl = h1T_s[:, b * 128:(b + 1) * 128]
                nc.vector.tensor_tensor(out=hsl, in0=a1[:], in1=a2[:],
                                        op=ALU.subtract)
                tp = tpp.tile([128, 128], BF16, tag="tp")
                nc.tensor.transpose(out=tp[:, 0:D], in_=hsl,
                                    identity=idm_s[:])
                nc.tensor.transpose(out=tp[:, D:2 * D], in_=hsl,
                                    identity=idm_s[:])
                hd = wp.tile([128, 2 * D], BF16, tag="hd")
                nc.scalar.activation(out=hd[:], in_=tp[:], func=ACTF.Copy)
                if b < NB // 2:
                    nc.sync.dma_start(out=h1dlA[b * 128:(b + 1) * 128, :],
                                      in_=hd[:])
                else:
                    bb = b - NB // 2
                    nc.sync.dma_start(out=h1dlB[bb * 128:(bb + 1) * 128, :],
                                      in_=hd[:])
                if debug_h1:
                    nc.sync.dma_start(
                        out=h1dbg_d[b * 128:(b + 1) * 128, :], in_=hd[:])

            def l2_tail(b, z):
                a1 = wp.tile([D, 128], BF16, tag="a1")
                nc.scalar.activation(out=a1[:], in_=z[:], func=ACTF.Relu,
                                     bias=b2_s[:, 0:1], scale=1.0)
                a2 = wp.tile([D, 128], BF16, tag="a2")
                nc.scalar.activation(out=a2[:], in_=z[:], func=ACTF.Relu,
                                     bias=nb2_s[:, 0:1], scale=-LEAKY)
                h2 = wp.tile([D, 128], BF16, tag="h2")
                nc.vector.tensor_tensor(out=h2[:], in0=a1[:], in1=a2[:],
                                        op=ALU.subtract)
                tp = tpp.tile([128, 128], BF16, tag="tp")
                nc.tensor.transpose(out=tp[:, 0:D], in_=h2[:],
                                    identity=idm_s[:])
                h2nm = wp.tile([128, D], BF16, tag="h2nm")
                nc.scalar.activation(out=h2nm[:], in_=tp[:, 0:D],
                                     func=ACTF.Copy)
                ph = ohp.tile([128, 128], BF16, tag="ph")
                nc.vector.tensor_scalar(out=ph[:], in0=iota_s[:],
                                        scalar1=gslot_s[:, b:b + 1],
                                        scalar2=None, op0=ALU.is_equal)
                nc.tensor.matmul(out=pool_ps[:], lhsT=ph[:], rhs=h2nm[:],
                                 start=(b == 0), stop=(b == NB - 1))

            def allgather(loc, full):
                nc.gpsimd.collective_compute(
                    "AllGather",
                    ALU.bypass,
                    replica_groups=[list(range(CORES))],
                    ins=[loc.ap()],
                    outs=[full.ap()],
                )

            for _rep in range(repeat):
                # ---- layer 1 (with AG_A after the first half) ----
                for g in range(NG if parts not in ("g", "gnc") else 0):
                    t, xt = fetch1(g)

                    def lhsT_l1(k, col, _t=t):
                        return _t[:, col * D:(col + 1) * D]

                    def xfm_l1(b, _xt=xt, _g=g):
                        return _xt[:, (b - _g * GROUP) * 128:
                                   (b - _g * GROUP + 1) * 128]

                    compute_group(meta1, g, dib1_s, wgt1_s, xfm_l1,
                                  w1r_s, w1o_s, l1_tail, lhsT_l1)
                    if g == NG // 2 - 1:
                        allgather(h1dlA, h1dfA)
                if parts == "g":
                    allgather(h1dlA, h1dfA)

                if parts == "l1":
                    po = wp.tile([128, D], F32, tag="pools")
                    nc.vector.tensor_copy(out=po[:], in_=dib2_s[:, 0:D])
                    nc.sync.dma_start(out=pool_d[:, :], in_=po[:])
                    continue
                # ---- layer 2, pipelined A/B gathers ----
                LOOK = 3
                tA, tB = {}, {}
                for g in range(min(LOOK - 1, NG)):
                    tA[g] = gather_half(g, 0)
                if parts != "gnc":
                    allgather(h1dlB, h1dfB)
                if LOOK - 1 < NG:
                    tA[LOOK - 1] = gather_half(LOOK - 1, 0)
                for g in range(NG):
                    if g + LOOK < NG:
                        tA[g + LOOK] = gather_half(g + LOOK, 0)
                    tB[g] = gather_half(g, 1)
                    handleA, offA = tA.pop(g)
                    handleB, offB = tB.pop(g)
                    if parts in ("l1g", "g", "gnc"):
                        continue

                    def lhsT_l2(k, col, _hA=handleA, _oA=offA,
                                _hB=handleB, _oB=offB):
                        if k < 2:
                            return _hA[:, _oA[k] + col, 0:D]
                        return _hB[:, _oB[k] + col, 0:D]

                    compute_group(
                        meta2, g, dib2_s, wgt2_s,
                        lambda b: h1T_s[:, b * 128:(b + 1) * 128],
                        w2r_s, w2o_s, l2_tail, lhsT_l2)

                if parts in ("l1g", "g", "gnc"):
                    po = wp.tile([128, D], F32, tag="pools")
                    nc.vector.tensor_copy(out=po[:], in_=dib2_s[:, 0:D])
                    nc.sync.dma_start(out=pool_d[:, :], in_=po[:])
                else:
                    pool_s = wp.tile([128, D], F32, tag="pools")
                    nc.scalar.activation(out=pool_s[:], in_=pool_ps[:],
                                         func=ACTF.Copy)
                    nc.sync.dma_start(out=pool_d[:, :], in_=pool_s[:])

    nc.compile()
    return nc


# ---------------------------------------------------------------------------
# Entry point
# ---------------------------------------------------------------------------

_CACHE = {}


def build(inputs, repeat=1, debug_h1=False, parts="full"):
    x = np.asarray(inputs["x_embeddings"], dtype=np.float32)
    in_maps, meta1, meta2, g_base = preprocess(
        x, inputs["edge_index"], inputs["weights"], inputs["batch"])
    common = _common_inputs(inputs["W1_root"], inputs["W1_rel"],
                            inputs["b1"], inputs["W2_root"],
                            inputs["W2_rel"], inputs["b2"])
    for m in in_maps:
        m.update(common)
    key = (meta1["C"], meta2["C"], repeat, debug_h1, parts)
    if key not in _CACHE:
        _CACHE[key] = build_nc(meta1, meta2, repeat=repeat,
                               debug_h1=debug_h1, parts=parts)
    return _CACHE[key], in_maps, g_base


def run(inputs, repeat=1, debug_h1=False):
    nc, in_maps, g_base = build(inputs, repeat, debug_h1)
    res = run_bass_kernel_spmd(nc, in_maps, core_ids=list(range(CORES)))

    batch = np.asarray(inputs["batch"], dtype=np.int64)
    counts = np.bincount(batch, minlength=NGRAPH).astype(np.float32)
    pooled = np.zeros((NGRAPH + 128, D), dtype=np.float32)
    for c in range(CORES):
        pooled[g_base[c]:g_base[c] + 128] += res.results[c]["pool"]
    pooled = pooled[:NGRAPH] / np.maximum(counts, 1.0)[:, None]
    out = pooled @ np.asarray(inputs["Wl_root"], dtype=np.float32)
    out = out + np.asarray(inputs["bl"], dtype=np.float32)
    return out.astype(np.float32), res


def kernel(**inputs) -> np.ndarray:
    out, _ = run(inputs)
    return out
